# revision 54
# baseline (speedup 1.0000x reference)
"""GCN (3-layer message passing) distributed over 8 TRN2 NeuronCores.

Sharding: nodes split evenly across 8 cores (rows). Weights replicated.
Per layer: local matmul h = x @ W + b (node-major out via x^T-stationary
matmuls), gather of the rows each peer needs into an AllToAll send
buffer, AllToAll exchange, then a local segment-sum implemented as
one-hot matmuls, with bias/relu fused in the epilogue.

v2 changes vs baseline:
- Host-side node permutation per core balances in-edges across the 50
  destination windows: every window gets exactly 4 edge tiles (512 edge
  slots), so the aggregation is a fixed 50x4 grid with ~2% padding.
  The output rows are un-permuted on the host after the kernel runs.
- One-hot matrices are precomputed on the host and streamed as static
  bf16 tensors (the graph is identical across layers, so one tensor is
  reused for all 3 layers), replacing per-tile vector is_equal ops.
- Aggregation gathers are batched 2 windows (1024 rows) per call and
  all runtime count registers are dropped: pad slots gather row 0 of
  the table (real, finite data) and hit an all-zero one-hot column.
- Matmuls use exact contraction (K = 1433/704/408 with a partial last
  128-block) and exact compute widths 704/408/16; DRAM tables stay at
  768/512/128 columns for the 256B-aligned gather row requirement.

Layer 3's inclusion linear Wi is folded into W3 (segment_sum commutes
with right-matmul), so the last exchange is only 16 (padded 128) wide.
"""
import sys

sys.path.insert(0, "/opt/trn_rl_repo")

import numpy as np
import ml_dtypes

import concourse.bass as bass
import concourse.bacc as bacc
import concourse.mybir as mybir
import concourse.tile as tile
from concourse.bass_utils import run_bass_kernel_spmd

NC = 8
BF16 = mybir.dt.bfloat16
F32 = mybir.dt.float32
I16 = mybir.dt.int16

N_NODES = 50000
NLOC = N_NODES // NC          # 6250
NW = 50                       # windows of 128 dst nodes per core
NPAD = NW * 128               # 6400
TPW = 4                       # edge tiles per window (512 slots)
T8 = NW * TPW                 # 200 tiles, 25600 edge slots per core
TWT = [768, 512, 128]         # DRAM h-table widths (gather 256B-aligned)
TWC = [704, 408, 16]          # compute widths (W1=700, W2=400, W3@Wi=16)
# contraction dims per layer: L2/L3 padded to full 128-blocks so the
# x^T DMA-transposes never hit the slow sub-tile fallback; the padded
# x columns are kept zero by pre-zeroed epilogue tiles.
KDIM = [1433, 768, 512]

last_exec_time_ns = None
last_results = None


def _wrap16(idx, ncols):
    """[n] int -> [128, n/16] int16 wrapped (idx i at [i%16, i//16]) and
    replicated to 128 partitions."""
    a = np.asarray(idx, np.int16).reshape(ncols, 16).T  # [16, n/16]
    return np.tile(a, (8, 1))


def _balance_windows(deg):
    """Assign NLOC nodes to NW windows of 128 nodes each s.t. every
    window's in-edge total is <= TPW*128. Greedy: heaviest nodes first
    into the least-loaded window with node capacity left.
    Returns prow[node] = window*128 + slot."""
    order = np.argsort(-deg, kind="stable")
    loads = np.zeros(NW, np.int64)
    counts = np.zeros(NW, np.int64)
    prow = np.empty(NLOC, np.int64)
    # iterate heaviest-first; pick least-loaded open window
    for n in order:
        w = None
        best = None
        for cand in np.argsort(loads, kind="stable"):
            if counts[cand] < 128:
                w = int(cand)
                break
        prow[n] = w * 128 + counts[w]
        counts[w] += 1
        loads[w] += deg[n]
    assert loads.max() <= TPW * 128, f"window overflow: {loads.max()}"
    return prow


def preprocess(features, W1, b1, W2, b2, W3, b3, Wi, bi, src, dst):
    """Host-side sharding/setup. Returns (cfg dict, in_maps list, unperm)."""
    N, K1 = features.shape  # 50000, 1433
    E = src.shape[0]

    # ---- weights (fold Wi into W3), padded, bf16 ----
    W3f = (W3.astype(np.float64) @ Wi.astype(np.float64)).astype(np.float32)
    b3f = (b3.astype(np.float64) @ Wi.astype(np.float64)).astype(np.float32)

    def pad2(a, r, c):
        out = np.zeros((r, c), np.float32)
        out[: a.shape[0], : a.shape[1]] = a
        return out

    # weight DRAM layout: [KB*128, TWC] (partial last K-block zero padded)
    KB = [(k + 127) // 128 for k in KDIM]
    w1 = pad2(W1, KB[0] * 128, TWC[0]).astype(ml_dtypes.bfloat16)
    w2 = pad2(W2, KB[1] * 128, TWC[1]).astype(ml_dtypes.bfloat16)
    w3 = pad2(W3f, KB[2] * 128, TWC[2]).astype(ml_dtypes.bfloat16)
    b1p = np.tile(pad2(b1[None, :], 1, TWC[0]), (128, 1))
    b2p = np.tile(pad2(b2[None, :], 1, TWC[1]), (128, 1))
    b3p = np.tile(pad2(b3f[None, :], 1, TWC[2]), (128, 1))
    bip = np.tile(pad2(bi[None, :], 1, TWC[2]), (128, 1))

    src = np.asarray(src, np.int64)
    dst = np.asarray(dst, np.int64)
    owner = src // NLOC
    dcore = dst // NLOC

    # ---- per-core window balancing permutation ----
    prows = []   # prows[c][local_node] = permuted row in [0, NPAD)
    for c in range(NC):
        ldst = dst[dcore == c] - c * NLOC
        deg = np.bincount(ldst, minlength=NLOC)
        prows.append(_balance_windows(deg))

    # ---- per-core transposed (permuted) features [K1, NPAD] bf16 ----
    featTs = []
    for c in range(NC):
        ft = np.zeros((K1, NPAD), np.float32)
        ft[:, prows[c]] = features[c * NLOC : (c + 1) * NLOC].T
        featTs.append(ft.astype(ml_dtypes.bfloat16))

    # ---- unique sources per (owner o -> dest d) pair, in permuted rows ----
    uniq = [[None] * NC for _ in range(NC)]
    for d in range(NC):
        maskd = dcore == d
        for o in range(NC):
            m = maskd & (owner == o)
            uniq[o][d] = np.unique(prows[o][src[m] - o * NLOC])
    P = max(len(uniq[o][d]) for o in range(NC) for d in range(NC))
    P = ((P + 127) // 128) * 128

    # ---- send-gather call plan: per dest, sorted rows in <=1024 chunks,
    # rebased to static [r0, r1) hloc slices (min/max over cores) so early
    # calls can start while the matmul still writes later rows.
    # The AllToAll is split SMALL-FIRST: part A = only the first 1024-row
    # chunk of every dest block (its rows exist ~40% into the matmul, so
    # collective A runs mostly under the matmul); part B = the remaining
    # chunks as one big collective right after. One ncfw gap, and the
    # bulk of the wire time starts ~120us earlier than a big-first split.
    # a2ain/recv layout: [NC*SA rows of A | NC*SB rows of B].
    ncalls = (P + 1023) // 1024
    csizes = [min(1024, P - k * 1024) for k in range(ncalls)]
    SA = 1024                     # rows per dest in part A (chunk 0)
    SB = P - SA                   # rows per dest in part B (chunks 1..)
    BOFF = NC * SA                # start of part B in a2ain/recv

    def a2apos(d, k):
        """a2ain row where (dest d, chunk k) starts."""
        return d * SA if k == 0 else BOFF + d * SB + (k - 1) * 1024

    NSC = NC * ncalls
    r0s = np.full(NSC, NPAD, np.int64)
    r1s = np.zeros(NSC, np.int64)
    for o in range(NC):
        for d in range(NC):
            u = uniq[o][d]
            for k, csz in enumerate(csizes):
                seg = u[k * 1024 : k * 1024 + csz]
                ci = d * ncalls + k
                if len(seg):
                    r0s[ci] = min(r0s[ci], seg[0])
                    r1s[ci] = max(r1s[ci], seg[-1] + 1)
    r0s = np.minimum(r0s, r1s)
    r1s = np.maximum(r1s, r0s + 1)
    scalls = []  # (k, pos0, csz, r0, r1)
    for d in range(NC):
        for k, csz in enumerate(csizes):
            ci = d * ncalls + k
            scalls.append((k, a2apos(d, k), csz, int(r0s[ci]), int(r1s[ci])))

    # send gather index stream per core o, laid out 1:1 with a2ain rows
    sidx = []
    for o in range(NC):
        stream = np.zeros(NC * P, np.int64)
        for d in range(NC):
            u = uniq[o][d]
            for k, csz in enumerate(csizes):
                ci = d * ncalls + k
                seg = u[k * 1024 : k * 1024 + csz] - r0s[ci]
                pos = a2apos(d, k)
                stream[pos : pos + len(seg)] = seg
        sidx.append(_wrap16(stream, NC * P // 16))

    # ---- aggregation: per dest core, edges into 50x4 tile grid ----
    gidxs, onehots = [], []
    for d in range(NC):
        m = dcore == d
        es, ed = src[m], dst[m]
        eo = es // NLOC
        pos = np.empty(len(es), np.int64)
        for o in range(NC):
            mo = eo == o
            pos[mo] = np.searchsorted(uniq[o][d], prows[o][es[mo] - o * NLOC])
        # recv-table row per edge (split A/B layout)
        tbl = np.where(pos < SA, eo * SA + pos, BOFF + eo * SB + (pos - SA))
        pd = prows[d][ed - d * NLOC]            # permuted dst row
        win = pd // 128
        rel = pd % 128
        g = np.zeros(T8 * 128, np.int64)        # pad -> row 0 (real data)
        oh = np.zeros((T8 * 128, 128), np.float32)  # pad -> all-zero col
        for w in range(NW):
            mw = win == w
            n = int(mw.sum())
            assert n <= TPW * 128
            off = w * TPW * 128
            order = np.argsort(tbl[mw], kind="stable")
            g[off : off + n] = tbl[mw][order]
            oh[np.arange(off, off + n), rel[mw][order]] = 1.0
        gidxs.append(_wrap16(g, T8 * 8))
        # onehot DRAM layout [128, T8, 128]: [slot-in-tile, tile, dstrel]
        onehots.append(np.ascontiguousarray(
            oh.reshape(T8, 128, 128).transpose(1, 0, 2)).astype(ml_dtypes.bfloat16))

    cfg = dict(P=P, scalls=scalls, NSC=NSC, KB=KB, SA=SA, SB=SB, BOFF=BOFF,
               ncalls=ncalls)

    bi2 = np.tile(pad2(bi[None, :], 1, 16), (128, NW))  # [128, NW*16]

    in_maps = []
    for c in range(NC):
        in_maps.append({
            "featT": featTs[c],
            "w1": w1, "w2": w2, "w3": w3,
            "b1": b1p, "b2": b2p, "b3": b3p, "bi2": bi2,
            "sidx": sidx[c], "gidx": gidxs[c], "onehot": onehots[c],
        })
    # output unscramble: row prow of core c's out = local node n
    unperm = np.empty(N, np.int64)
    for c in range(NC):
        unperm[c * NLOC : (c + 1) * NLOC] = c * NPAD + prows[c]
    return cfg, in_maps, unperm


def build(cfg, nq=4):
    P, scalls, NSC, KB = cfg["P"], cfg["scalls"], cfg["NSC"], cfg["KB"]
    SA, SB, BOFF, ncalls = cfg["SA"], cfg["SB"], cfg["BOFF"], cfg["ncalls"]
    OUT_W = 16

    nc = bacc.Bacc("TRN2", target_bir_lowering=False, debug=False,
                   num_devices=NC, num_swdge_queues=nq)

    featT = nc.declare_dram_parameter("featT", [KDIM[0], NPAD], BF16, isOutput=False)
    wts = [nc.declare_dram_parameter(f"w{l+1}", [KB[l] * 128, TWC[l]], BF16,
                                     isOutput=False) for l in range(3)]
    bs = [nc.declare_dram_parameter(f"b{l+1}", [128, TWC[l]], F32, isOutput=False)
          for l in range(3)]
    bi2 = nc.declare_dram_parameter("bi2", [128, NW * OUT_W], F32, isOutput=False)
    sidx = nc.declare_dram_parameter("sidx", [128, NC * P // 16], I16, isOutput=False)
    gidx = nc.declare_dram_parameter("gidx", [128, T8 * 8], I16, isOutput=False)
    onehot = nc.declare_dram_parameter("onehot", [128, T8, 128], BF16, isOutput=False)
    out = nc.declare_dram_parameter("out", [NPAD, OUT_W], F32, isOutput=True)

    hloc = [nc.dram_tensor(f"hloc{l}", [NPAD, TWT[l]], BF16) for l in range(3)]
    a2ain = [nc.dram_tensor(f"a2ain{l}", [NC * P, TWT[l]], BF16) for l in range(3)]
    recv = [nc.dram_tensor(f"recv{l}", [NC * P, TWT[l]], BF16) for l in range(3)]
    xs = [None, nc.dram_tensor("x2", [NPAD, KDIM[1]], BF16),
          nc.dram_tensor("x3", [NPAD, KDIM[2]], BF16)]

    groups = [list(range(NC))]

    with tile.TileContext(nc) as tc:
        with (
            tc.tile_pool(name="wpool", bufs=1) as wpool,
            tc.tile_pool(name="bpool", bufs=1) as bpool,
            tc.tile_pool(name="ipool", bufs=1) as ipool,
            tc.tile_pool(name="xtp", bufs=2) as xtp,
            tc.tile_pool(name="mmpsum", bufs=2, space="PSUM") as mmpsum,
            tc.tile_pool(name="hbp", bufs=3) as hbp,
            tc.tile_pool(name="sgp", bufs=5) as sgp,
            tc.tile_pool(name="agp", bufs=3) as agp,
            tc.tile_pool(name="ohp", bufs=3) as ohp,
            tc.tile_pool(name="apsum", bufs=2, space="PSUM") as apsum,
            tc.tile_pool(name="xop", bufs=3) as xop,
        ):
            # resident: indices
            sidx_t = ipool.tile([128, NC * P // 16], I16, tag="sidx")
            nc.sync.dma_start(sidx_t[:], sidx[:])
            gidx_t = ipool.tile([128, T8 * 8], I16, tag="gidx")
            nc.sync.dma_start(gidx_t[:], gidx[:])
            # pre-zero the x epilogue tiles: the [TWC, KDIM) pad columns of
            # x2/x3 must read as 0 in the next layer's contraction
            for lz in (1, 2):
                for _ in range(3):
                    zt = xop.tile([128, KDIM[lz]], BF16, tag=f"xo{lz-1}")
                    nc.vector.memset(zt[:], 0.0)

            for l in range(3):
              TW, TC, K = TWT[l], TWC[l], KDIM[l]
              kb_n = KB[l]
              klast = K - (kb_n - 1) * 128    # rows in the last K block
              nslices = [(s, min(s + 512, TC)) for s in range(0, TC, 512)]
              with nc.named_scope(f"L{l}"):
                  # ---- resident weights/bias for this layer ----
                  wt = wpool.tile([128, kb_n, TC], BF16, tag="w")
                  nc.sync.dma_start(
                      wt[:], wts[l].rearrange("(kb p) w -> p kb w", p=128))
                  bt = bpool.tile([128, TC], F32, tag="b")
                  nc.sync.dma_start(bt[:], bs[l][:])
                  if l == 2:
                      bit = bpool.tile([128, NW * OUT_W], F32, tag="bi2")
                      nc.sync.dma_start(bit[:], bi2[:])

                  # ---- matmul: h = x @ W + b  (node-major PSUM out) ----
                  sc_mm = nc.enter_named_scope(f"mm{l}", False)[0]
                  NRW = 1024
                  for nr in range(0, NPAD, NRW):
                      rw = min(NRW, NPAD - nr)
                      stripes = []
                      for kb in range(kb_n):
                          kr = 128 if kb < kb_n - 1 else klast
                          st = xtp.tile([128, NRW], BF16, tag=f"xt{kb}")
                          if l == 0:
                              nc.sync.dma_start(
                                  st[:kr, :rw],
                                  featT[kb * 128 : kb * 128 + kr, nr : nr + rw])
                          else:
                              # scalar (Activation) HWDGE queue: keeps the
                              # transpose issue cost off the busy sync queue
                              nc.scalar.dma_start_transpose(
                                  st[:kr, :rw],
                                  xs[l][nr : nr + rw, kb * 128 : kb * 128 + kr])
                          stripes.append(st)
                      for m in range(rw // 128):
                          ps = mmpsum.tile([128, TC], F32, tag="mmps")
                          for kb in range(kb_n):
                              kr = 128 if kb < kb_n - 1 else klast
                              for (s0, s1) in nslices:
                                  nc.tensor.matmul(
                                      ps[:, s0:s1],
                                      stripes[kb][:kr, m * 128 : (m + 1) * 128],
                                      wt[:kr, kb, s0:s1],
                                      start=(kb == 0), stop=(kb == kb_n - 1))
                          hb = hbp.tile([128, TW], BF16, tag="hb")
                          nc.vector.tensor_tensor(
                              hb[:, :TC], ps[:], bt[:], op=mybir.AluOpType.add)
                          nc.sync.dma_start(
                              hloc[l][nr + m * 128 : nr + (m + 1) * 128, :TC],
                              hb[:, :TC])

                  nc.leave_named_scope(f"mm{l}", sc_mm, False)
                  # ---- send gather + split exchange ----
                  sc_sg = nc.enter_named_scope(f"sg{l}", False)[0]

                  def emit_sg(calls):
                      # k-major order: calls needing only early h rows first
                      for i, (k, pos0, csz, r0, r1) in enumerate(calls):
                          g = sgp.tile([128, 8, TW], BF16, tag="sg")
                          nb = csz // 128
                          nc.gpsimd.dma_gather(
                              g[:, :nb, :], hloc[l][r0:r1],
                              sidx_t[:, pos0 // 16 : (pos0 + csz) // 16],
                              csz, csz, TW, queue_num=i % nq)
                          # during mm0 the scalar HWDGE queue is idle (no
                          # transposes for l=0): stores there complete
                          # sooner, so the collectives trigger earlier
                          eng = nc.scalar if l == 0 else nc.sync
                          eng.dma_start(
                              a2ain[l][pos0 : pos0 + csz, :]
                              .rearrange("(b p) w -> p b w", p=128),
                              g[:, :nb, :])

                  callsA = sorted((c for c in scalls if c[0] == 0),
                                  key=lambda t: t[1])
                  callsB = sorted((c for c in scalls if c[0] >= 1),
                                  key=lambda t: (t[0], t[1]))
                  emit_sg(callsA)
                  # collective A covers chunk 0 of every dest block; it
                  # launches mid-matmul and overlaps the chunk-B gathers
                  nc.gpsimd.collective_compute(
                      "AllToAll", mybir.AluOpType.bypass, replica_groups=groups,
                      ins=[a2ain[l][:BOFF]], outs=[recv[l][:BOFF]])
                  emit_sg(callsB)
                  nc.gpsimd.collective_compute(
                      "AllToAll", mybir.AluOpType.bypass, replica_groups=groups,
                      ins=[a2ain[l][BOFF:]], outs=[recv[l][BOFF:]])
                  nc.leave_named_scope(f"sg{l}", sc_sg, False)
                  # ---- aggregation: segment-sum via one-hot matmuls ----
                  sc_ag = nc.enter_named_scope(f"agg{l}", False)[0]
                  if l == 2:
                      # one wide PSUM holds all 50 window outputs (16 cols
                      # each); single fused bias+relu pass at the end
                      ps3 = apsum.tile([128, NW * OUT_W], F32, tag="aps")
                  for wb in range(0, NW, 2):  # 2 windows (1024 rows) per batch
                      t0 = wb * TPW
                      gt = agp.tile([128, 2 * TPW, TW], BF16, tag="ag")
                      nc.gpsimd.dma_gather(
                          gt[:], recv[l][:],
                          gidx_t[:, t0 * 8 : (t0 + 2 * TPW) * 8],
                          2 * TPW * 128, 2 * TPW * 128, TW,
                          queue_num=(wb // 2) % nq)
                      oht = ohp.tile([128, 2 * TPW, 128], BF16, tag="oh")
                      nc.scalar.dma_start(oht[:], onehot[:, t0 : t0 + 2 * TPW, :])
                      for wi in range(2):
                          w = wb + wi
                          if l == 2:
                              for tl in range(TPW):
                                  b = wi * TPW + tl
                                  nc.tensor.matmul(
                                      ps3[:, w * OUT_W : (w + 1) * OUT_W],
                                      oht[:, b, :], gt[:, b, :OUT_W],
                                      start=(tl == 0), stop=(tl == TPW - 1))
                              continue
                          ps = apsum.tile([128, NW * OUT_W], F32, tag="aps")
                          for tl in range(TPW):
                              b = wi * TPW + tl
                              for (s0, s1) in nslices:
                                  nc.tensor.matmul(
                                      ps[:, s0:s1], oht[:, b, :],
                                      gt[:, b, s0:s1],
                                      start=(tl == 0), stop=(tl == TPW - 1))
                          xb = xop.tile([128, KDIM[l + 1]], BF16, tag=f"xo{l}")
                          nc.vector.tensor_scalar_max(
                              xb[:, :TC], ps[:, :TC], 0.0)
                          nc.sync.dma_start(
                              xs[l + 1][w * 128 : (w + 1) * 128, :], xb[:])
                  if l == 2:
                      ob = xop.tile([128, NW * OUT_W], F32, tag="ob")
                      nc.vector.tensor_tensor(
                          ob[:], ps3[:], bit[:], op=mybir.AluOpType.add)
                      nc.vector.tensor_scalar_max(ob[:], ob[:], 0.0)
                      nc.sync.dma_start(
                          out[:].rearrange("(w p) c -> p w c", p=128),
                          ob[:].rearrange("p (w c) -> p w c", c=OUT_W))
                  nc.leave_named_scope(f"agg{l}", sc_ag, False)
    nc.finalize()
    return nc


def kernel(**inputs):
    global last_exec_time_ns, last_results
    inputs = {k: np.asarray(v) for k, v in inputs.items()}
    cfg, in_maps, unperm = preprocess(**inputs)
    nc = build(cfg)
    res = None
    # trace=True needs the axon NTFF hook; fall back to untraced runs, and
    # retry once more on transient device errors.
    for attempt, trace in enumerate([True, False, False]):
        try:
            res = run_bass_kernel_spmd(
                nc, in_maps, core_ids=list(range(NC)), trace=trace)
            break
        except Exception:
            if attempt == 2:
                raise
            import time
            time.sleep(15)
    last_exec_time_ns = res.exec_time_ns
    last_results = res
    allrows = np.concatenate([res.results[c]["out"] for c in range(NC)], axis=0)
    return np.ascontiguousarray(allrows[unperm])
